# revision 32
# baseline (speedup 1.0000x reference)
"""Multi-head attention (B=4, S=2048, E=1024, H=16) on 8 trn2 NeuronCores.

Sharding: data-parallel over B (4) x tensor-parallel over H (2 halves of 8
heads). Core c handles batch c//2, head-half c%2. Column-parallel qkv_proj,
row-parallel out_proj; the all-reduce of the two partial outputs per batch
(and the bout bias add) is done on the host during unshard. The device emits
out^T [E, S] to keep DMA writes contiguous.

Device kernel (identical program on all 8 cores):
  qk-proj  bf16 matmuls -> psum; DVE quantizes to fp8e4 as q_hi / q_lo
           (lo = exact residual) and k (duplicated via sbuf-sbuf DMA).
  scores   fp8e4 DoubleRow matmuls: the two DR slots carry (q_hi, q_lo)
           against a duplicated k, so full hi/lo precision on q at 0.5
           cycles/row; contraction K=64 on partitions 0-63 / 64-127 per head.
  exp      split across ACT (table exp, ~80%) and DVE (degree-6 monic
           Horner polynomial in bf16 at 4x DVE perf mode, ~20%); e in bf16.
  PV       bf16, out[q, d] orientation: lhsT = e tile (stationary),
           rhs = [v | 1] so the softmax denominator rides in column 64.
  norm     DVE reciprocal + per-partition scale -> ctx bf16.
  ctx^T    PE transpose-mode matmuls (bf16) + Pool evict -> ctxT sbuf.
  out-proj bf16 matmuls per 512-seq strip (strips 0-2 overlap the tail of
           attention), DVE evict, DMA out.
"""
import sys

import numpy as np

sys.path.insert(0, "/opt/trn_rl_repo")

import concourse.bacc as bacc
import concourse.mybir as mybir
import concourse.tile as tile
from concourse.bass_utils import run_bass_kernel_spmd

F32 = mybir.dt.float32
BF16 = mybir.dt.bfloat16
F8 = mybir.dt.float8e4
EXP = mybir.ActivationFunctionType.Exp
DR = mybir.MatmulPerfMode.DoubleRow
ADD = mybir.AluOpType.add
MULT = mybir.AluOpType.mult
SUB = mybir.AluOpType.subtract

B, S, E, H, HD = 4, 2048, 1024, 16, 64
HL = 8              # heads per core
SCALE = 1.0 / 32.0  # 1/sqrt(E)

# degree-6 monic Horner coefficients for exp(s/32) on |s| <= 73:
# e ~= C * (((((s+b5)s+b4)s+b3)s+b2)s+b1)s + b0-step folded into final ts
POLY_C = 1.0517495615029438e-12
POLY_B = [951276856510.5004, 29695717017.265644, 461030552.26193637,
          4804592.785561899, 39941.03240420765, 269.9281676602873]

# fraction of exp chunks handed to DVE: every Nth chunk (large = ACT only)
DVE_EVERY = 10 ** 9


def build_nc():
    nc = bacc.Bacc("TRN2", target_bir_lowering=False, debug=False, num_devices=8)
    xT_d = nc.declare_dram_parameter("xT", [E, S], BF16, isOutput=False)
    wqk_d = nc.declare_dram_parameter("wqk", [E, 1024], BF16, isOutput=False)
    wv_d = nc.declare_dram_parameter("wv", [E, 512], BF16, isOutput=False)
    wo_d = nc.declare_dram_parameter("wo", [512, E], BF16, isOutput=False)
    bqk_d = nc.declare_dram_parameter("bqk", [128, 8], F32, isOutput=False)
    cons_d = nc.declare_dram_parameter("cons", [128, 768], BF16, isOutput=False)
    out_d = nc.declare_dram_parameter("outT", [E, S], F32, isOutput=True)

    with tile.TileContext(nc) as tc:
      with tc.tile_pool(name="pm", bufs=1) as pm:
        # ---- persistent sbuf tensors
        ident_s = pm.tile([128, 128], BF16)        # identity for PE transpose
        onesbv_s = pm.tile([1, 768], BF16)         # [0:128] ones, [256:768] bv
        bqk_s = pm.tile([128, 8], F32)
        xT_s = pm.tile([128, 8, S], BF16)          # x^T by E-chunk
        wv_s = pm.tile([128, 8, 512], BF16)
        wo_s = pm.tile([128, 4, E], BF16)
        qf8 = pm.tile([128, 4, 2, S], F8)          # [part, hp, hi/lo, seq]
        kf8 = pm.tile([128, 4, 2, S], F8)          # [part, hp, dup, seq]
        v1_s = pm.tile([128, 16, HL, 65], BF16)    # [keys, kt, head, v|1]
        ctxT_s = pm.tile([128, 4, S], BF16)        # [d-part, dgroup, seq]

        nc.sync.dma_start(out=ident_s, in_=cons_d[:, 0:128])
        nc.sync.dma_start(out=onesbv_s, in_=cons_d[0:1, :])
        nc.sync.dma_start(out=bqk_s, in_=bqk_d[:, :])
        # warm the ACT exp table early (load is ~1.3us)
        warm = pm.tile([1, 1], F32)
        nc.scalar.activation(out=warm, in_=bqk_s[0:1, 0:1], func=EXP)
        # ones column of v1
        nc.gpsimd.memset(v1_s[:, :, :, 64:65], 1.0)

        def load_xT(ic):
            nc.sync.dma_start(
                out=xT_s[:, :, ic * 512:(ic + 1) * 512],
                in_=xT_d[:, ic * 512:(ic + 1) * 512]
                .rearrange("(c p) s -> p c s", p=128))

        with tc.tile_pool(name="pw", bufs=1) as pw, \
             tc.tile_pool(name="ps", bufs=1, space="PSUM") as ps, \
             tc.tile_pool(name="pt", bufs=1) as pt:

            # ---------------- emission helpers ----------------
            def load_wqk(hp):
                w = pw.tile([128, 8, 256], BF16, tag="wqk", bufs=2,
                            name=f"wqk{hp}")
                nc.scalar.dma_start(
                    out=w,
                    in_=wqk_d[:, hp * 256:(hp + 1) * 256]
                    .rearrange("(c p) d -> p c d", p=128))
                return w

            def qk_piece(hp, w, t, ic):
                # one [128 dims, 512 seq] tile of q (t=0) or k (t=1) + quant
                pq = ps.tile([128, 512], F32, tag="qkv", bufs=1, name="pq")
                for ec in range(8):
                    nc.tensor.matmul(
                        out=pq, lhsT=w[:, ec, t * 128:(t + 1) * 128],
                        rhs=xT_s[:, ec, ic * 512:(ic + 1) * 512],
                        start=(ec == 0), stop=(ec == 7))
                sl = slice(ic * 512, (ic + 1) * 512)
                bsl = bqk_s[:, 2 * hp + t:2 * hp + t + 1]
                if t == 0:
                    nc.vector.tensor_scalar_add(
                        out=qf8[:, hp, 0, sl], in0=pq, scalar1=bsl)
                    nc.vector.scalar_tensor_tensor(
                        out=qf8[:, hp, 1, sl], in0=pq, scalar=bsl,
                        in1=qf8[:, hp, 0, sl], op0=ADD, op1=SUB)
                else:
                    nc.vector.tensor_scalar_add(
                        out=kf8[:, hp, 0, sl], in0=pq, scalar1=bsl)
                    # slot duplication on the (otherwise idle) Pool engine;
                    # keeps the startup path off the busy SP DMA queue
                    nc.gpsimd.tensor_copy(kf8[:, hp, 1, sl],
                                          kf8[:, hp, 0, sl])

            def v_piece(kt):
                # one [128 keys, 512 vdims] tile of v + bias + evict to v1
                pv = ps.tile([128, 512], F32, tag="qkv", bufs=1, name="pv")
                for ec in range(8):
                    nc.tensor.matmul(
                        out=pv, lhsT=xT_s[:, ec, kt * 128:(kt + 1) * 128],
                        rhs=wv_s[:, ec, :], start=(ec == 0), stop=False)
                nc.tensor.matmul(
                    out=pv, lhsT=onesbv_s[0:1, 0:128],
                    rhs=onesbv_s[0:1, 256:768], start=False, stop=True)
                nc.vector.tensor_copy(
                    v1_s[:, kt, :, 0:64],
                    pv.rearrange("p (h d) -> p h d", d=64))

            chunk_n = [0]

            def sc_chunk(h, q5, ck, e_g):
                hp, pb = h // 2, (h % 2) * 64
                sc = ps.tile([128, 2, 512], F32, tag="sc", bufs=2, name="sc")
                for k2 in range(2):
                    kt = ck * 2 + k2
                    for qi in range(2):
                        qsl = slice(q5 * 512 + qi * 256, q5 * 512 + (qi + 1) * 256)
                        nc.tensor.matmul(
                            out=sc[:, k2, qi * 256:(qi + 1) * 256],
                            lhsT=kf8[pb:pb + 64, hp, :, kt * 128:(kt + 1) * 128],
                            rhs=qf8[pb:pb + 64, hp, :, qsl],
                            perf_mode=DR, start=True, stop=True)
                esl = e_g[:, ck * 2:ck * 2 + 2, :]
                if chunk_n[0] % DVE_EVERY == DVE_EVERY - 1:
                    sbf = pt.tile([128, 1024], BF16, tag="sbf", bufs=2)
                    y = pt.tile([128, 1024], BF16, tag="y", bufs=2)
                    scv = sc.rearrange("p a b -> p (a b)")
                    nc.vector.tensor_copy(sbf, scv)
                    nc.vector.scalar_tensor_tensor(
                        out=y, in0=sbf, scalar=POLY_B[5], in1=sbf,
                        op0=ADD, op1=MULT)
                    for k in (4, 3, 2, 1):
                        nc.vector.scalar_tensor_tensor(
                            out=y, in0=y, scalar=POLY_B[k], in1=sbf,
                            op0=ADD, op1=MULT)
                    nc.vector.tensor_scalar(
                        out=esl.rearrange("p a b -> p (a b)"), in0=y,
                        scalar1=POLY_B[0], scalar2=POLY_C, op0=ADD, op1=MULT)
                else:
                    nc.scalar.activation(out=esl, in_=sc, func=EXP,
                                         scale=float(SCALE))
                chunk_n[0] += 1

            def finish_group(h, q5, e_g):
                hp, pb = h // 2, (h % 2) * 64
                pvt = ps.tile([128, 4, 65], F32, tag="pvacc", bufs=2, name="pvt")
                for j in range(4):
                    for kt in range(16):
                        nc.tensor.matmul(
                            out=pvt[:, j, :],
                            lhsT=e_g[:, kt, j * 128:(j + 1) * 128],
                            rhs=v1_s[:, kt, h, :],
                            start=(kt == 0), stop=(kt == 15))
                rr = pt.tile([128, 4], F32, tag="rr", bufs=2)
                rs = pt.tile([128, 4], F32, tag="rs", bufs=2)
                nc.vector.reciprocal_approx_accurate(
                    out=rr, in_=pvt[:, :, 64], scratch=rs)
                ctx_t = pt.tile([128, 4, 64], BF16, tag="ctx", bufs=2)
                for j in range(4):
                    nc.vector.tensor_scalar_mul(
                        out=ctx_t[:, j, :], in0=pvt[:, j, 0:64],
                        scalar1=rr[:, j:j + 1])
                tx = ps.tile([64, 4, 128], BF16, tag="tx", bufs=1, name="tx")
                for j in range(4):
                    nc.tensor.matmul(out=tx[:, j, :], lhsT=ctx_t[:, j, :],
                                     rhs=ident_s, is_transpose=True)
                nc.vector.tensor_copy(
                    ctxT_s[pb:pb + 64, hp, q5 * 512:(q5 + 1) * 512],
                    tx.rearrange("p a b -> p (a b)"))

            def out_strip_mono(qc):
                for et in range(8):
                    po = ps.tile([128, 512], F32, tag="qkv", bufs=1,
                                 name=f"pom{qc}_{et}")
                    for t in range(4):
                        nc.tensor.matmul(
                            out=po,
                            lhsT=wo_s[:, t, et * 128:(et + 1) * 128],
                            rhs=ctxT_s[:, t, qc * 512:(qc + 1) * 512],
                            start=(t == 0), stop=(t == 3))
                    ot = pt.tile([128, 512], F32, tag="ot", bufs=2)
                    nc.vector.tensor_copy(ot, po)
                    nc.sync.dma_start(
                        out=out_d[et * 128:(et + 1) * 128,
                                  qc * 512:(qc + 1) * 512], in_=ot)

            def out_strip_partial(qc):
                # dgroups 0-2 (heads 0-5) of the strip, accumulated to sbuf
                # bf16 so only the 1-matmul dgroup-3 tail waits for head 7
                acc = pt.tile([128, 8, 512], BF16, tag="oacc", bufs=2,
                              name=f"oacc{qc}")
                for et in range(8):
                    po = ps.tile([128, 512], F32, tag="qkv", bufs=1,
                                 name=f"pop{qc}_{et}")
                    for t in range(3):
                        nc.tensor.matmul(
                            out=po,
                            lhsT=wo_s[:, t, et * 128:(et + 1) * 128],
                            rhs=ctxT_s[:, t, qc * 512:(qc + 1) * 512],
                            start=(t == 0), stop=(t == 2))
                    nc.vector.tensor_copy(acc[:, et, :], po)
                return acc

            def out_strip_final(qc, acc):
                for et in range(8):
                    # at this point the scores ping/pong ring is drained, so
                    # borrow it to pipeline matmul vs. the DVE add
                    po = ps.tile([128, 2, 512], F32, tag="sc", bufs=2,
                                 name=f"po{qc}_{et}")
                    nc.tensor.matmul(
                        out=po[:, 0, :],
                        lhsT=wo_s[:, 3, et * 128:(et + 1) * 128],
                        rhs=ctxT_s[:, 3, qc * 512:(qc + 1) * 512],
                        start=True, stop=True)
                    ot = pt.tile([128, 512], F32, tag="ot", bufs=2)
                    nc.vector.tensor_tensor(ot, po[:, 0, :], acc[:, et, :],
                                            ADD)
                    nc.sync.dma_start(
                        out=out_d[et * 128:(et + 1) * 128,
                                  qc * 512:(qc + 1) * 512], in_=ot)

            # ---------------- emission schedule ----------------
            # DMA order matters: first qk-proj piece needs xT[ic] + wqk0;
            # wv/wo/later xT chunks follow behind on the same queue.
            wqk_cur = load_wqk(0)
            load_xT(0)
            for ic in range(1, 4):
                load_xT(ic)
            nc.sync.dma_start(
                out=wv_s, in_=wv_d[:, :].rearrange("(c p) d -> p c d", p=128))
            nc.sync.dma_start(
                out=wo_s, in_=wo_d[:, :].rearrange("(c p) e -> p c e", p=128))
            # minimal prologue: first scores only need q(ic=0) + all k pieces
            # of hp0; the remaining q pieces aren't needed until the q5=1
            # sweep, so they drip as filler.
            qk_piece(0, wqk_cur, 0, 0)
            qk_piece(0, wqk_cur, 1, 0)

            # fillers: remaining q pieces of hp0 first (q ic_n is needed by
            # group n), then v pieces (needed by the first finish_group,
            # which is deferred 2 groups); next hp's qk pieces get prepended
            # during the h-odd q5=0 group.
            fill = []                      # list of zero-arg emitters
            for ic in range(1, 4):
                # k ic_n feeds chunk 2n of group 0; q ic_n feeds group n
                fill.append(lambda ic=ic: qk_piece(0, wqk_cur, 1, ic))
                fill.append(lambda ic=ic: qk_piece(0, wqk_cur, 0, ic))
            for kt in range(16):
                fill.append(lambda kt=kt: v_piece(kt))

            # finishes lag 2 groups early on (so the dripped v pieces can
            # drain first), then catch up to lag 1 to keep the tail short
            e_tiles = {}
            str_acc = {}
            fin = 0            # next group index to finish
            groups = [(h, q5) for h in range(8) for q5 in range(4)]
            for g, (h, q5) in enumerate(groups):
                if q5 == 0 and h % 2 == 1 and h // 2 + 1 < 4:
                    # next head-pair's qk pieces must drain before group g+4
                    hpn = h // 2 + 1
                    wn = load_wqk(hpn)
                    fill[0:0] = [
                        (lambda hpn=hpn, wn=wn, t=t, ic=ic:
                         qk_piece(hpn, wn, t, ic))
                        for t in range(2) for ic in range(4)]
                e_g = pm.tile([128, 16, 512], BF16, tag="e", bufs=4,
                              name=f"e{g}")
                e_tiles[g] = e_g
                for ck in range(8):
                    sc_chunk(h, q5, ck, e_g)
                    if fill:
                        fill.pop(0)()
                target = (g - 2) if g < 8 else (g - 1)
                while fin <= target:
                    ph, pq5 = groups[fin]
                    finish_group(ph, pq5, e_tiles.pop(fin))
                    if ph == 5 and pq5 >= 2:
                        str_acc[pq5] = out_strip_partial(pq5)
                    elif ph == 7:
                        if pq5 >= 2:
                            out_strip_final(pq5, str_acc.pop(pq5))
                        else:
                            out_strip_mono(pq5)
                    fin += 1
            while fill:
                fill.pop(0)()
            while fin < 32:
                ph, pq5 = groups[fin]
                finish_group(ph, pq5, e_tiles.pop(fin))
                if ph == 5 and pq5 >= 2:
                    str_acc[pq5] = out_strip_partial(pq5)
                elif ph == 7:
                    if pq5 >= 2:
                        out_strip_final(pq5, str_acc.pop(pq5))
                    else:
                        out_strip_mono(pq5)
                fin += 1

    nc.compile()
    return nc


_NC = None


def _get_nc():
    global _NC
    if _NC is None:
        _NC = build_nc()
    return _NC


def make_in_maps(query, Wqkv, bqkv, Wout, bout):
    import ml_dtypes
    query = np.asarray(query, dtype=np.float32)
    Wqkv = np.asarray(Wqkv, dtype=np.float32)
    bqkv = np.asarray(bqkv, dtype=np.float32)
    Wout = np.asarray(Wout, dtype=np.float32)

    def bf(x):
        return np.ascontiguousarray(np.asarray(x, dtype=ml_dtypes.bfloat16))

    in_maps = []
    for c in range(8):
        b, hh = c // 2, c % 2
        heads = np.arange(hh * HL, hh * HL + HL)
        dims = (heads[:, None] * HD + np.arange(HD)[None, :]).reshape(-1)

        # wqk cols: hp*256 + t*128 + d128 (d128 = dims of heads 2hp, 2hp+1)
        wqk = np.empty((E, 1024), np.float32)
        bqk = np.empty((128, 8), np.float32)
        for hp in range(4):
            d128 = dims[hp * 128:(hp + 1) * 128]
            wqk[:, hp * 256:hp * 256 + 128] = Wqkv[d128].T
            wqk[:, hp * 256 + 128:hp * 256 + 256] = Wqkv[E + d128].T
            bqk[:, 2 * hp] = bqkv[d128]
            bqk[:, 2 * hp + 1] = bqkv[E + d128]

        wv = Wqkv[2 * E + dims].T                     # [E, 512]
        wo = Wout[:, dims].T                          # [512, E]

        cons = np.zeros((128, 768), np.float32)
        cons[:, 0:128] = np.eye(128, dtype=np.float32)
        cons[0, 128:256] = 1.0
        cons[0, 256:768] = bqkv[2 * E + dims]

        in_maps.append({
            "xT": bf(query[b].T), "wqk": bf(wqk), "wv": bf(wv), "wo": bf(wo),
            "bqk": np.ascontiguousarray(bqk), "cons": bf(cons),
        })
    return in_maps


def gather(results, bout=None):
    out = np.empty((B, S, E), np.float32)
    for b in range(B):
        acc = results[2 * b]["outT"] + results[2 * b + 1]["outT"]   # [E, S]
        out[b] = acc.T
    if bout is not None:
        out += np.asarray(bout, dtype=np.float32)
    return out


def kernel(query, key, value, Wqkv, bqkv, Wout, bout):
    # key/value are unused by the reference module (qkv all from query)
    nc = _get_nc()
    in_maps = make_in_maps(query, Wqkv, bqkv, Wout, bout)
    res = run_bass_kernel_spmd(nc, in_maps, list(range(8)))
    return gather(res.results, bout)
